# revision 17
# baseline (speedup 1.0000x reference)
"""Trainium2 Bass kernel for nn_CollapsedPBFAOptimized (Chebyshev kernelized
linear attention).

Sharding (8 cores): core c handles batch b = c//4 and the 4 heads
[4*(c%4) .. 4*(c%4)+3].  Each core computes a partial output
(x[b] @ w_in_sub -> features -> per-head KV state -> out rows) projected
through its w_out columns; the host sums the 4 partials per batch.

Math: with T_m Chebyshev polynomials and C[m,p] their power-basis
coefficients, sum_m beta_m T_m(q)T_m(k) = sum_{p,r} q^p G[p,r] k^r with
G = C^T diag(beta) C.  G is computed on the host from the actual beta;
when its support is confined to powers <= 5 (true for the collapsed
beta, whose tail coefficients vanish) only powers 0..5 are materialized
on chip (npow=6), else the full 0..10 set is used (npow=11).

Device pipeline (per core, bf16 matmul operands / fp32 PSUM):
  ph1  fused QKV projection from x (clamped q,k evicted as bf16)
  ph2  k-power planes (ones,k^1..) + per-head state S[v',(r,d)] via PE
       accumulation over all 32 s-chunks (merged into the ph1 loop when
       npow=6 so elementwise work hides under the projection matmuls)
  ph3  transpose state to [(r,d),v'], apply G (x) I_64 via small PE
       matmuls, build parity-swapped copy + p=0 bias column
  ph4  q-power planes, q-side einsum into PSUM with the p=0 term added
       as a per-partition bias at eviction, then the w_out projection,
       streamed out per 512-token chunk as bf16.
"""
import json
import sys
import numpy as np
from contextlib import ExitStack
from functools import lru_cache

sys.path.insert(0, '/opt/trn_rl_repo')

import ml_dtypes
import concourse.bass as bass
import concourse.tile as tile
from concourse import mybir, bass_utils

# ---------------------------------------------------------------------------
# Toolchain patches
# ---------------------------------------------------------------------------


def _install_patches():
    """This walrus build supports only ONE sync-wait command per instruction.
    (a) Split the TileContext tail drain's waits across multiple Drains.
    (b) Post-process the BIR JSON: hoist excess on_wait entries onto injected
        NoOps on the same engine (engine program order makes this equivalent;
        for queue DMAs the trigger write is the ordering point)."""
    from concourse.tile import ScopedClock
    from concourse import bass2jax

    def _patched_drain_and_barrier(self, tick_clock, wait_clock):
        drain_inst = self.nc.sync.drain()
        wait_clock.add_sem_waits(
            drain_inst.ins, ScopedClock({None: tick_clock.global_clock}))
        si = drain_inst.ins.sync_info
        if si is not None:
            w = list(si.on_wait)
            if len(w) > 1:
                si.on_wait = [w[0]]
                for extra in w[1:]:
                    d2 = self.nc.sync.drain()
                    d2.ins.sync_info = mybir.SyncInfo(on_wait=[extra], on_update=[])
        self.nc.all_engine_barrier()
        assert self.sems is not None
        popped = self.nc._tile_sem_poison_stack.pop()
        assert popped is self._sem_poison
        self.nc.clear_and_free_semaphores(list(self.sems.allocated().values()))
        self.nc.all_engine_barrier()

    tile.TileContext._drain_and_barrier = _patched_drain_and_barrier

    LIMIT = 1

    def split_waits_in_bir_json(bir_json):
        d = json.loads(bir_json.decode() if isinstance(bir_json, bytes) else bir_json)
        for fn in d.get('functions', []):
            for bb in fn.get('blocks', []):
                out, changed = [], False
                for ins in bb.get('instructions', []):
                    si = ins.get('sync_info')
                    waits = (si or {}).get('on_wait') or []
                    if len(waits) > LIMIT:
                        for k, w in enumerate(waits[:-LIMIT]):
                            nop = {'name': ins['name'] + f'-xw{k}',
                                   'engine': ins['engine'], 'opcode': 'NoOp',
                                   'ins': [], 'outs': [],
                                   'sync_info': {'on_wait': [w], 'on_update': []}}
                            if 'debug' in ins:
                                nop['debug'] = ins['debug']
                            out.append(nop)
                        si['on_wait'] = waits[-LIMIT:]
                        changed = True
                    out.append(ins)
                if changed:
                    bb['instructions'] = out
        return json.dumps(d).encode()

    if not getattr(bass_utils.compile_bir_kernel, '_wait_patched', False):
        orig = bass_utils.compile_bir_kernel

        def patched(bir_json, tmpdir, neff_name='file.neff'):
            return orig(split_waits_in_bir_json(bir_json), tmpdir, neff_name)

        patched._wait_patched = True
        bass_utils.compile_bir_kernel = patched
        bass2jax.compile_bir_kernel = patched


_install_patches()

# ---------------------------------------------------------------------------
# Problem constants (hardcoded per the task contract)
# ---------------------------------------------------------------------------
B, S, D = 2, 4096, 1024
H, DH = 16, 64
M = 11                       # max Chebyshev order span (degrees 0..10)
SCALE = DH ** -0.5
HPC = 4                      # heads per core
NCORES = 8
F32 = mybir.dt.float32
BF16 = mybir.dt.bfloat16
TS = mybir.AluOpType
SQ = mybir.ActivationFunctionType.Square
IDENT = mybir.ActivationFunctionType.Identity


def _cheb_C():
    C = np.zeros((M, M), dtype=np.float64)
    for m in range(M):
        e = np.zeros(m + 1)
        e[m] = 1.0
        c = np.polynomial.chebyshev.cheb2poly(e) if m > 0 else np.array([1.0])
        C[m, :len(c)] = c
    return C


# ---------------------------------------------------------------------------
# Device program
# ---------------------------------------------------------------------------


def _build_program(npow, shared):
    NJ = (npow + 1) // 2          # r/p pair-chunk count (3 or 6)
    NBLK = NJ * NJ if shared else HPC * NJ * NJ
    nc = bass.Bass('TRN2', target_bir_lowering=False, debug=False,
                   num_devices=NCORES)
    ap = {}
    ap['xP'] = nc.dram_tensor('xP', (128, 8, S), BF16, kind='ExternalInput').ap()
    ap['wq'] = nc.dram_tensor('wq', (128, 8, 256), BF16, kind='ExternalInput').ap()
    ap['wkv'] = nc.dram_tensor('wkv', (128, 8, 512), BF16, kind='ExternalInput').ap()
    ap['wo'] = nc.dram_tensor('wo', (128, 2, D), BF16, kind='ExternalInput').ap()
    ap['gb'] = nc.dram_tensor('gb', (128, NBLK, 128), BF16, kind='ExternalInput').ap()
    ap['eye'] = nc.dram_tensor('eye', (64, 64), F32, kind='ExternalInput').ap()
    ap['outp'] = nc.dram_tensor('outp', (128, 8, S), BF16, kind='ExternalOutput').ap()

    with tile.TileContext(nc) as tc:
        with ExitStack() as ctx:
            _emit(nc, tc, ctx, ap, npow, NJ, shared)
    return nc


def _emit(nc, tc, ctx, ap, npow, NJ, shared):
    NP1 = npow                     # planes in KP: ones + k^1..k^(npow-1)
    merged = (npow == 6)

    persist = ctx.enter_context(tc.tile_pool(name='persist', bufs=1))
    xp = ctx.enter_context(tc.tile_pool(name='xp', bufs=2))

    # x chunk 0 first so the first projection matmul isn't stuck behind the
    # full weight-upload on the single DMA queue; wkv before wq because the
    # kv-projection matmuls are emitted first
    def fetch_x(ss):
        xss = xp.tile([128, 8 * 512], BF16, tag='xss', name='xss')
        x3d = xss[:].rearrange("p (i c) -> p i c", c=512)
        src = ap['xP'][:, :, ss * 512:(ss + 1) * 512]
        nc.sync.dma_start(x3d[:, 0:4, :], src[:, 0:4, :])
        nc.sync.dma_start(x3d[:, 4:8, :], src[:, 4:8, :])
        return xss

    x_pref = xp.tile([128, 8 * 512], BF16, tag='xss', name='xss0')
    xp3d = x_pref[:].rearrange("p (i c) -> p i c", c=512)
    src0 = ap['xP'][:, :, 0:512]
    nc.sync.dma_start(xp3d[:, 0:4, :], src0[:, 0:4, :])
    wkv_sb = persist.tile([128, 8 * 512], BF16, tag='wkv', name='wkv_sb')
    nc.sync.dma_start(wkv_sb[:].rearrange("p (i c) -> p i c", c=512), ap['wkv'][:])
    nc.sync.dma_start(xp3d[:, 4:8, :], src0[:, 4:8, :])
    wq_sb = persist.tile([128, 8 * 256], BF16, tag='wq', name='wq_sb')
    nc.sync.dma_start(wq_sb[:].rearrange("p (i c) -> p i c", c=256), ap['wq'][:])

    eye_sb = persist.tile([64, 64], F32, tag='eye', name='eye_sb')
    ones_bf = persist.tile([128, 64], BF16, tag='ones', name='ones_bf')
    nc.gpsimd.memset(ones_bf[:], 1.0)
    wo_sb = persist.tile([128, 2 * D], BF16, tag='wo', name='wo_sb')
    NBLK = ap['gb'].shape[1]
    gb_sb = persist.tile([128, NBLK * 128], BF16, tag='gb', name='gb_sb')

    def fetch_late_weights():
        # queued behind the x DMAs; lands long before ph3/ph4 need it
        nc.sync.dma_start(eye_sb[:], ap['eye'][:])
        nc.sync.dma_start(wo_sb[:].rearrange("p (i c) -> p i c", c=D), ap['wo'][:])
        nc.sync.dma_start(gb_sb[:].rearrange("p (i c) -> p i c", c=128), ap['gb'][:])

    qb = [persist.tile([128, S], BF16, tag=f'qb{hp}', name=f'qb{hp}')
          for hp in range(2)]
    stv = [persist.tile([64, NP1 * 64], F32, tag=f'stv{h}', name=f'stv{h}')
           for h in range(HPC)]
    # full-sequence q-power planes, built inside the ph1 loop (merged path)
    # so phase 4 is left with only matmuls + evictions
    QPf = {}
    if merged:
        for hp in range(2):
            for p in range(2, npow):
                QPf[(p, hp)] = persist.tile(
                    [128, S], BF16, tag=f'qf{p}_{hp}', name=f'qf{p}_{hp}')

    wq3 = wq_sb[:].rearrange("p (i c) -> p i c", c=256)
    wkv3 = wkv_sb[:].rearrange("p (i c) -> p i c", c=512)
    wo3 = wo_sb[:].rearrange("p (i c) -> p i c", c=D)
    gb3 = gb_sb[:].rearrange("p (i c) -> p i c", c=128)

    # builds k-power planes 2..npow-1 of a KP tile from plane 1 (plane 0 is
    # ones, set once at allocation; plane 1 is written by the clamp eviction
    # in the merged path).  kp4: [128, plane, chunk, 64].  Leaf squares go
    # to the otherwise-idle GpSimd engine.
    def build_planes(kp4):
        nc.scalar.activation(kp4[:, 2], kp4[:, 1], SQ)
        nc.vector.tensor_tensor(kp4[:, 3], kp4[:, 2], kp4[:, 1], op=TS.mult)
        nc.vector.tensor_tensor(kp4[:, 5], kp4[:, 2], kp4[:, 3], op=TS.mult)
        if NP1 > 6:
            nc.scalar.activation(kp4[:, 4], kp4[:, 2], SQ)
            nc.scalar.activation(kp4[:, 6], kp4[:, 3], SQ)
            nc.vector.tensor_tensor(kp4[:, 7], kp4[:, 3], kp4[:, 4], op=TS.mult)
            nc.gpsimd.tensor_tensor(kp4[:, 8], kp4[:, 4], kp4[:, 4], op=TS.mult)
            nc.vector.tensor_tensor(kp4[:, 9], kp4[:, 4], kp4[:, 5], op=TS.mult)
            nc.scalar.activation(kp4[:, 10], kp4[:, 5], SQ)
        else:
            nc.gpsimd.tensor_tensor(kp4[:, 4], kp4[:, 2], kp4[:, 2], op=TS.mult)

    # ---------------- Phases 1+2: QKV projection + KV state ---------------
    # npow=6: single merged loop (state psum fits alongside projection psum:
    # 2+2+4 = 8 banks).  npow=11: projection first, then a separate state
    # pass (needs 8 state banks by itself).  kv projection is emitted before
    # q so the last chunk's state matmuls aren't stuck behind the q matmuls.
    def proj_ss(ss, xss, ps1, kevict, vevict):
        x3 = xss[:].rearrange("p (i c) -> p i c", c=512)
        for sc in range(4):
            pkv = ps1.tile([128, 512], F32, tag='pkv', name='pkv')
            for i in range(8):
                nc.tensor.matmul(pkv[:], x3[:, i, sc * 128:(sc + 1) * 128],
                                 wkv3[:, i], start=(i == 0), stop=(i == 7))
            nc.vector.tensor_scalar(kevict(sc), pkv[:, 0:256].rearrange(
                "p (h w) -> p h w", w=64), -1.0, 1.0, op0=TS.max, op1=TS.min)
            vevict(sc, pkv)
        for hp in range(2):
            pq = ps1.tile([128, 512], F32, tag='pq', name='pq')
            for i in range(8):
                nc.tensor.matmul(pq[:], wq3[:, i, hp * 128:(hp + 1) * 128],
                                 x3[:, i], start=(i == 0), stop=(i == 7))
            nc.vector.tensor_scalar(qb[hp][:, ss * 512:(ss + 1) * 512], pq[:],
                                    -1.0, 1.0, op0=TS.max, op1=TS.min)

    if merged:
        with tc.tile_pool(name='kvp', bufs=2) as kvp, \
             tc.tile_pool(name='kpp', bufs=1) as kpp, \
             tc.tile_pool(name='ps1', bufs=2, space='PSUM') as ps1, \
             tc.tile_pool(name='pst', bufs=1, space='PSUM') as pstp:
            kp_t = []
            for j in range(2):
                t = kpp.tile([128, HPC * NP1 * 256], BF16, tag=f'kp{j}',
                             name=f'kp{j}')
                k5 = t[:].rearrange("p (h r c w) -> p h r c w",
                                    h=HPC, r=NP1, c=4)
                for h in range(HPC):
                    nc.gpsimd.memset(k5[:, h, 0, :, :], 1.0)   # ones plane
                kp_t.append(k5)
            st_ps = [pstp.tile([64, NP1 * 64], F32, tag=f'st{h}', name=f'st{h}')
                     for h in range(HPC)]
            fetched = x_pref
            for ss in range(8):
                xss = fetched
                if ss < 7:
                    fetched = fetch_x(ss + 1)
                if ss == 0:
                    fetch_late_weights()
                kp5 = kp_t[ss % 2]
                vc = kvp.tile([128, 4 * 256], BF16, tag='vc', name='vc')
                proj_ss(ss, xss, ps1,
                        lambda sc: kp5[:, :, 1, sc, :],
                        lambda sc, pkv: nc.scalar.copy(
                            vc[:, sc * 256:(sc + 1) * 256], pkv[:, 256:512]))
                for hp in range(2):
                    w = slice(ss * 512, (ss + 1) * 512)
                    qs = qb[hp][:, w]
                    nc.scalar.activation(QPf[(2, hp)][:, w], qs, SQ)
                    nc.vector.tensor_tensor(QPf[(3, hp)][:, w],
                                            QPf[(2, hp)][:, w], qs, op=TS.mult)
                    nc.gpsimd.tensor_tensor(QPf[(4, hp)][:, w],
                                            QPf[(2, hp)][:, w],
                                            QPf[(2, hp)][:, w], op=TS.mult)
                    nc.vector.tensor_tensor(QPf[(5, hp)][:, w],
                                            QPf[(2, hp)][:, w],
                                            QPf[(3, hp)][:, w], op=TS.mult)
                for h in range(HPC):
                    build_planes(kp5[:, h])
                    for cl in range(4):
                        c = ss * 4 + cl
                        nc.tensor.matmul(
                            st_ps[h][:], vc[:, cl * 256 + h * 64:cl * 256 + (h + 1) * 64],
                            kp5[:, h, :, cl, :], start=(c == 0), stop=(c == 31))
            for h in range(HPC):
                if h % 2 == 0:
                    nc.scalar.copy(stv[h][:], st_ps[h][:])
                else:
                    nc.vector.tensor_copy(stv[h][:], st_ps[h][:])
    else:
        kv_ctx = ExitStack()
        kv_pool = kv_ctx.enter_context(tc.tile_pool(name='kv', bufs=1))
        k_all = kv_pool.tile([128, 32 * 256], BF16, tag='k_all', name='k_all')
        v_all = kv_pool.tile([128, 32 * 256], BF16, tag='v_all', name='v_all')
        with tc.tile_pool(name='ps1', bufs=2, space='PSUM') as ps1:
            fetched = x_pref
            for ss in range(8):
                xss = fetched
                if ss < 7:
                    fetched = fetch_x(ss + 1)
                if ss == 0:
                    fetch_late_weights()
                koff = ss * 1024
                proj_ss(ss, xss, ps1,
                        lambda sc: k_all[:, koff + sc * 256:koff + (sc + 1) * 256]
                        .rearrange("p (h w) -> p h w", w=64),
                        lambda sc, pkv: nc.scalar.copy(
                            v_all[:, koff + sc * 256:koff + (sc + 1) * 256],
                            pkv[:, 256:512]))
        # separate state pass: per head two psum groups (planes 0..5, 6..10)
        k3 = k_all[:].rearrange("p (c w) -> p c w", w=256)
        with tc.tile_pool(name='kpp', bufs=1) as kpp, \
             tc.tile_pool(name='pst', bufs=1, space='PSUM') as pstp:
            kp_t = []
            for j in range(2):
                t = kpp.tile([128, NP1 * 512], BF16, tag=f'kp{j}', name=f'kp{j}')
                nc.gpsimd.memset(t[:, 0:512], 1.0)
                kp_t.append(t)
            stA = [pstp.tile([64, 384], F32, tag=f'stA{h}', name=f'stA{h}')
                   for h in range(HPC)]
            stB = [pstp.tile([64, (NP1 - 6) * 64], F32, tag=f'stB{h}',
                             name=f'stB{h}') for h in range(HPC)]
            for h in range(HPC):
                for g in range(4):
                    kp = kp_t[g % 2]
                    kp4 = kp[:].rearrange("p (r c w) -> p r c w", c=8, w=64)
                    nc.scalar.copy(kp4[:, 1],
                                   k3[:, 8 * g:8 * (g + 1), h * 64:(h + 1) * 64])
                    build_planes(kp4)
                    for cl in range(8):
                        c = 8 * g + cl
                        vsl = v_all[:, c * 256 + h * 64:c * 256 + (h + 1) * 64]
                        nc.tensor.matmul(stA[h][:], vsl, kp4[:, 0:6, cl, :],
                                         start=(c == 0), stop=(c == 31))
                        nc.tensor.matmul(stB[h][:], vsl, kp4[:, 6:NP1, cl, :],
                                         start=(c == 0), stop=(c == 31))
            for h in range(HPC):
                nc.scalar.copy(stv[h][:, 0:384], stA[h][:])
                nc.scalar.copy(stv[h][:, 384:NP1 * 64], stB[h][:])
        kv_ctx.close()

    # ---------------- Phase 3: transpose + G transform + bias -------------
    stT = persist.tile([128, NJ * HPC * 64], BF16, tag='stT', name='stT')
    stT4 = stT[:].rearrange("p (j h w) -> p j h w", h=HPC, w=64)
    Wsb = persist.tile([128, NJ * HPC * 64], BF16, tag='Wsb', name='Wsb')
    Wsw = persist.tile([128, NJ * HPC * 64], BF16, tag='Wsw', name='Wsw')
    Wsb4 = Wsb[:].rearrange("p (j h w) -> p j h w", h=HPC, w=64)
    Wsw4 = Wsw[:].rearrange("p (j h w) -> p j h w", h=HPC, w=64)
    c0sb = persist.tile([128, 2], F32, tag='c0sb', name='c0sb')
    if npow % 2 == 1:                     # kill garbage in the half chunk
        nc.gpsimd.memset(stT4[64:128, NJ - 1, :, :], 0.0)

    with tc.tile_pool(name='psT', bufs=4, space='PSUM') as psTp, \
         tc.tile_pool(name='psW', bufs=2, space='PSUM') as psWp, \
         tc.tile_pool(name='psc', bufs=2, space='PSUM') as pscp:
        n = 0
        for h in range(HPC):
            for r in range(NP1):
                j, rl = r // 2, r % 2
                pT = psTp.tile([64, 64], F32, tag='pT', name='pT')
                nc.tensor.transpose(pT[:], stv[h][:, r * 64:(r + 1) * 64],
                                    eye_sb[:])
                dst = stT4[rl * 64:(rl + 1) * 64, j, h, :]
                if n % 2 == 0:
                    nc.scalar.copy(dst, pT[:])
                else:
                    nc.vector.tensor_copy(dst, pT[:])
                n += 1
        if shared:
            for i in range(NJ):
                pW = psWp.tile([128, HPC * 64], F32, tag='pW', name='pW')
                for j in range(NJ):
                    nc.tensor.matmul(pW[:], gb3[:, i * NJ + j, :],
                                     stT4[:, j, :, :],
                                     start=(j == 0), stop=(j == NJ - 1))
                nc.scalar.copy(Wsb4[:, i, :, :], pW[:])
        else:
            for h in range(HPC):
                for i in range(NJ):
                    pW = psWp.tile([128, HPC * 64], F32, tag='pW', name='pW')
                    for j in range(NJ):
                        nc.tensor.matmul(pW[:, 0:64],
                                         gb3[:, (h * NJ + i) * NJ + j, :],
                                         stT4[:, j, h, :],
                                         start=(j == 0), stop=(j == NJ - 1))
                    nc.scalar.copy(Wsb4[:, i, h, :], pW[:, 0:64])
        for i in range(NJ):
            nc.vector.tensor_copy(Wsw4[0:64, i, :, :], Wsb4[64:128, i, :, :])
            nc.vector.tensor_copy(Wsw4[64:128, i, :, :], Wsb4[0:64, i, :, :])
        for hp in range(2):
            pc0 = pscp.tile([128, 1], F32, tag='pc0', name='pc0')
            nc.tensor.matmul(pc0[:], Wsb4[0:64, 0, 2 * hp:2 * hp + 2, :],
                             ones_bf[0:64, 0:1], start=True, stop=True)
            nc.scalar.copy(c0sb[:, hp:hp + 1], pc0[:])

    # ---------------- Phase 4: q-side einsum + output projection ----------
    out3 = ap['outp']
    with tc.tile_pool(name='qpp', bufs=4) as qpp, \
         tc.tile_pool(name='otp', bufs=2) as otp, \
         tc.tile_pool(name='obp', bufs=2) as obp, \
         tc.tile_pool(name='ps4', bufs=3, space='PSUM') as ps4, \
         tc.tile_pool(name='ps5', bufs=4, space='PSUM') as ps5:
        for t in range(8):
            oT = []
            for hp in range(2):
                qsl = qb[hp][:, t * 512:(t + 1) * 512]
                if merged:
                    QP = {p: QPf[(p, hp)][:, t * 512:(t + 1) * 512]
                          for p in range(2, npow)}
                else:
                    QP = {p: qpp.tile([128, 512], BF16, tag=f'q{p}',
                                      name=f'q{p}')[:]
                          for p in range(2, npow)}
                    nc.scalar.activation(QP[2], qsl, SQ)
                    nc.vector.tensor_tensor(QP[3], QP[2], qsl, op=TS.mult)
                    nc.gpsimd.tensor_tensor(QP[4], QP[2], QP[2], op=TS.mult)
                    nc.vector.tensor_tensor(QP[5], QP[2], QP[3], op=TS.mult)
                    nc.scalar.activation(QP[6], QP[3], SQ)
                    nc.vector.tensor_tensor(QP[7], QP[3], QP[4], op=TS.mult)
                    nc.gpsimd.tensor_tensor(QP[8], QP[4], QP[4], op=TS.mult)
                    nc.vector.tensor_tensor(QP[9], QP[4], QP[5], op=TS.mult)
                    nc.scalar.activation(QP[10], QP[5], SQ)
                pO = ps4.tile([128, 512], F32, tag='pO', name='pO')
                # role 0 and role 1 matmuls use disjoint PE quadrants
                # (rows/cols 0:64 vs 64:128) — interleave so they overlap
                for p in range(1, npow):
                    i, pl = p // 2, p % 2
                    for role in range(2):
                        h = 2 * hp + role
                        lo, hi = role * 64, (role + 1) * 64
                        if role == 0:
                            wsrc = Wsb4 if pl == 0 else Wsw4
                        else:
                            wsrc = Wsw4 if pl == 0 else Wsb4
                        lhsT = wsrc[lo:hi, i, h, :]
                        rhs = qsl[lo:hi, :] if p == 1 else QP[p][lo:hi, :]
                        nc.tensor.matmul(pO[lo:hi, :], lhsT, rhs,
                                         start=(p == 1), stop=(p == npow - 1))
                ot = otp.tile([128, 512], BF16, tag='oT', name='oT')
                nc.scalar.activation(ot[:], pO[:], IDENT,
                                     bias=c0sb[:, hp:hp + 1])
                oT.append(ot)
            ob = obp.tile([128, 8 * 512], BF16, tag='ob', name='ob')
            for o in range(8):
                pP = ps5.tile([128, 512], F32, tag='pP', name='pP')
                nc.tensor.matmul(pP[:], wo3[:, 0, o * 128:(o + 1) * 128],
                                 oT[0][:], start=True, stop=False)
                nc.tensor.matmul(pP[:], wo3[:, 1, o * 128:(o + 1) * 128],
                                 oT[1][:], start=False, stop=True)
                dst = ob[:, o * 512:(o + 1) * 512]
                if o % 2 == 0:
                    nc.scalar.copy(dst, pP[:])
                else:
                    nc.vector.tensor_copy(dst, pP[:])
            nc.sync.dma_start(out3[:, :, t * 512:(t + 1) * 512],
                              ob[:].rearrange("p (o c) -> p o c", c=512))


@lru_cache(maxsize=2)
def _get_program(npow, shared):
    return _build_program(npow, shared)


# ---------------------------------------------------------------------------
# Host-side packing
# ---------------------------------------------------------------------------


def _g_blocks(G, npow, shared):
    """Pack G (x) I_64 pair-blocks: block (i,j) is the [128,128] matrix
    lhsT[(rl,d),(pl,d')] = G[2i+pl, 2j+rl] * delta(d,d')."""
    NJ = (npow + 1) // 2
    dd = np.eye(64, dtype=np.float64)
    heads = range(1) if shared else range(HPC)
    blocks = []
    for h in heads:
        for i in range(NJ):
            for j in range(NJ):
                blk = np.zeros((128, 128), dtype=np.float64)
                for pl in range(2):
                    p = 2 * i + pl
                    if p >= npow:
                        continue
                    for rl in range(2):
                        r = 2 * j + rl
                        if r >= npow:
                            continue
                        blk[rl * 64:(rl + 1) * 64, pl * 64:(pl + 1) * 64] = \
                            G[h, p, r] * dd
                blocks.append(blk)
    arr = np.stack(blocks).astype(ml_dtypes.bfloat16)     # (NBLK,128,128)
    return np.ascontiguousarray(arr.transpose(1, 0, 2))   # (128,NBLK,128)


last_results = None


def kernel(x, w_in, w_out, beta):
    x = np.asarray(x, dtype=np.float32)
    w_in = np.asarray(w_in, dtype=np.float32)
    w_out = np.asarray(w_out, dtype=np.float32)
    beta = np.asarray(beta, dtype=np.float32)

    C = _cheb_C()
    G = np.einsum('mp,hm,mr->hpr', C, beta.astype(np.float64), C)  # (H,11,11)
    npow = 6 if (np.abs(G[:, 6:, :]).max() == 0.0
                 and np.abs(G[:, :, 6:]).max() == 0.0) else 11
    shared = bool(np.all(beta == beta[0:1]))
    nc = _get_program(npow, shared)

    eye = np.eye(64, dtype=np.float32)
    # xP[b][p, i, s] = x[b][s, i*128+p]
    xP = [np.ascontiguousarray(
            x[b].T.reshape(8, 128, S).transpose(1, 0, 2)
          ).astype(ml_dtypes.bfloat16) for b in range(B)]

    in_maps = []
    for core in range(NCORES):
        b, hg = core // 4, core % 4
        heads = [4 * hg + j for j in range(HPC)]
        wqT = np.empty((D, 256), dtype=np.float32)
        wkvT = np.empty((D, 512), dtype=np.float32)
        woT = np.empty((256, D), dtype=np.float32)
        for hl, h in enumerate(heads):
            wqT[:, hl * 64:(hl + 1) * 64] = (SCALE * w_in[h * DH:(h + 1) * DH, :]).T
            wkvT[:, hl * 64:(hl + 1) * 64] = (SCALE * w_in[D + h * DH:D + (h + 1) * DH, :]).T
            wkvT[:, 256 + hl * 64:256 + (hl + 1) * 64] = w_in[2 * D + h * DH:2 * D + (h + 1) * DH, :].T
            woT[hl * 64:(hl + 1) * 64, :] = w_out[:, h * DH:(h + 1) * DH].T
        gb = _g_blocks(G[heads] if not shared else G[:1], npow, shared)
        in_maps.append({
            'xP': xP[b],
            'wq': np.ascontiguousarray(
                wqT.reshape(8, 128, 256).transpose(1, 0, 2)).astype(ml_dtypes.bfloat16),
            'wkv': np.ascontiguousarray(
                wkvT.reshape(8, 128, 512).transpose(1, 0, 2)).astype(ml_dtypes.bfloat16),
            'wo': np.ascontiguousarray(
                woT.reshape(2, 128, D).transpose(1, 0, 2)).astype(ml_dtypes.bfloat16),
            'gb': gb,
            'eye': eye,
        })

    res = bass_utils.run_bass_kernel_spmd(nc, in_maps, core_ids=list(range(NCORES)))
    global last_results
    last_results = res

    out = np.zeros((B, S, D), dtype=np.float32)
    for core in range(NCORES):
        o = np.asarray(res.results[core]['outp']).astype(np.float32)
        # o[p, i, s] -> rows i*128+p
        out[core // 4] += o.transpose(1, 0, 2).reshape(D, S).T
    return out
